# revision 13
# baseline (speedup 1.0000x reference)
"""Bass/Trainium2 kernel for masked dot-product attention (B=32, L=8192, D=128).

Computation (matches the reference):
    score[b,l] = sum_d(query[b,0,d] * context[b,l,d]) / scale
    score      = mask ? -BIG : score
    weight     = softmax(score, axis=l)             -> [B, L, 1]
    z[b,d]     = sum_l weight[b,l] * context[b,l,d] -> [B, D]
returns (weight, z).

Sharding: data-parallel over batch, 4 batches per core x 8 cores.

Per-core device plan (all of the core's context lives in SBUF: 16 MiB):
  - context tile layout [128(p=l//64), 4(i=batch), 64(r=l%64), 128(d)]\n    (partition-contiguous rows -> 32 KB DMA descriptors)
  - scores: per (i,t) one fused DVE scalar_tensor_tensor:
        S[p,(i,t)] = sum_d ctx*q   (+ mask penalty added per batch)
  - softmax without max-subtraction (|score| <~ 80 -> exp safe in fp32);
    exp+row-partial-sum fused in one ScalarE activation per batch,
    partition-sum / reciprocal / partition-broadcast via tiny PE matmuls
  - z: 64 accumulating PE matmuls per batch into PSUM [128(d),1],
    scaled by 1/sum on ScalarE at the end.
Outputs land as w[128,4,64] / z[128,4]; the host transposes back (cheap).
"""

import os
import sys

import numpy as np

B, L, D = 32, 8192, 128
NCORES = 8
BPC = B // NCORES  # batches per core
NT = L // 128  # 128-row L-tiles per batch
PEN = np.float32(-1e30)

TRACE = False  # test harness sets kernel.TRACE = True for profiling
LAST_RESULT = None  # BassKernelResults of the last run (for the harness)

_CACHE = {}


def _ensure_concourse():
    try:
        import concourse  # noqa: F401
        return
    except ImportError:
        pass
    for p in ("/opt/trn_rl_repo", "/root/.axon_site/_ro/trn_rl_repo"):
        if os.path.isdir(p) and p not in sys.path:
            sys.path.insert(0, p)
    import concourse  # noqa: F401


def _build_bass(repeat=1):
    key = ("nc", repeat)
    if key in _CACHE:
        return _CACHE[key]
    _ensure_concourse()
    import concourse.bacc as bacc
    import concourse.tile as tile
    from concourse import mybir

    f32 = mybir.dt.float32

    nc = bacc.Bacc("TRN2", target_bir_lowering=False, debug=False, num_devices=NCORES)

    ctx_d = nc.dram_tensor("ctx", [BPC, L, D], f32, kind="ExternalInput").ap()
    qb_d = nc.dram_tensor("qb", [128, BPC, D], f32, kind="ExternalInput").ap()
    pen_d = nc.dram_tensor("pen", [128, BPC, NT], f32, kind="ExternalInput").ap()
    w_d = nc.dram_tensor("w_out", [128, BPC, NT], f32, kind="ExternalOutput").ap()
    z_d = nc.dram_tensor("z_out", [128, BPC], f32, kind="ExternalOutput").ap()

    with tile.TileContext(nc) as tc:
        with (
            tc.tile_pool(name="big", bufs=1) as big,
            tc.tile_pool(name="small", bufs=1) as small,
            tc.tile_pool(name="zps", bufs=4, space="PSUM") as zps,
            tc.tile_pool(name="sps", bufs=2, space="PSUM") as sps,
        ):
            ctx_sb = big.tile([128, BPC, NT, D], f32)
            qb_sb = small.tile([128, BPC, D], f32)
            pen_sb = small.tile([128, BPC, NT], f32)
            # per-batch tiles: avoids whole-tile WAR serialization between
            # batch i's readers (PE z-matmuls) and batch i+1's STT writes
            S_i = [small.tile([128, NT], f32, name=f"S{i}", tag=f"S{i}") for i in range(BPC)]
            w_i = [small.tile([128, NT], f32, name=f"w{i}", tag=f"w{i}") for i in range(BPC)]
            prod = small.tile([128, D], f32)  # STT elementwise-product sink
            part = small.tile([128, BPC], f32)  # per-partition exp sums
            inv_sb = small.tile([128, BPC], f32)  # 1/sum broadcast per batch
            inv_row = small.tile([1, BPC], f32)
            z_sb = small.tile([128, BPC], f32)
            ones_col = small.tile([128, 1], f32)
            ones_row = small.tile([1, 128], f32)

            nc.vector.memset(ones_col, 1.0)
            nc.vector.memset(ones_row, 1.0)

            nc.sync.dma_start(out=qb_sb, in_=qb_d)
            nc.sync.dma_start(out=pen_sb, in_=pen_d)

            ctx_r = ctx_d.rearrange("i (p r) d -> p i r d", p=128)
            CH = 16
            for _rep in range(repeat):
                # context rows p*64+r -> SBUF [p, i, r, d]; contiguous 32 KB
                # per (partition, batch): 512 big descriptors total
                for i in range(BPC):
                    for tq in range(NT // CH):
                        sl = slice(tq * CH, (tq + 1) * CH)
                        nc.sync.dma_start(
                            out=ctx_sb[:, i, sl, :], in_=ctx_r[:, i, sl, :]
                        )

                for i in range(BPC):
                    # -- scores: S[p, i, t] = sum_d ctx*q (fused mult+reduce) --
                    for t in range(NT):
                        nc.vector.scalar_tensor_tensor(
                            out=prod,
                            in0=ctx_sb[:, i, t, :],
                            scalar=1.0,
                            in1=qb_sb[:, i, :],
                            op0=mybir.AluOpType.mult,
                            op1=mybir.AluOpType.mult,
                            accum_out=S_i[i][:, t : t + 1],
                        )
                    # -- add mask penalty (gpsimd: keeps DVE streaming) --
                    nc.gpsimd.tensor_add(
                        out=S_i[i], in0=S_i[i], in1=pen_sb[:, i, :]
                    )
                    # -- exp + per-partition partial sum (fused) --
                    nc.scalar.activation(
                        out=S_i[i],
                        in_=S_i[i],
                        func=mybir.ActivationFunctionType.Exp,
                        accum_out=part[:, i : i + 1],
                    )
                    # -- total = ones.T @ part (partition reduce) --
                    s_ps = sps.tile([1, 1], f32)
                    nc.tensor.matmul(
                        out=s_ps, lhsT=ones_col, rhs=part[:, i : i + 1],
                        start=True, stop=True,
                    )
                    nc.vector.reciprocal(out=inv_row[:, i : i + 1], in_=s_ps)
                    # -- broadcast 1/total to all 128 partitions --
                    ib_ps = sps.tile([128, 1], f32)
                    nc.tensor.matmul(
                        out=ib_ps, lhsT=ones_row, rhs=inv_row[:, i : i + 1],
                        start=True, stop=True,
                    )
                    nc.scalar.copy(out=inv_sb[:, i : i + 1], in_=ib_ps)
                    # -- z (unnormalized): accumulate over tiles on PE --
                    z_ps = zps.tile([128, 1], f32)
                    for t in range(NT):
                        nc.tensor.matmul(
                            out=z_ps,
                            lhsT=ctx_sb[:, i, t, :],
                            rhs=S_i[i][:, t : t + 1],
                            start=(t == 0),
                            stop=(t == NT - 1),
                        )
                    # -- normalize (gpsimd: keeps DVE streaming) --
                    nc.gpsimd.tensor_scalar_mul(
                        out=w_i[i], in0=S_i[i],
                        scalar1=inv_sb[:, i : i + 1],
                    )
                    nc.scalar.mul(
                        out=z_sb[:, i : i + 1], in_=z_ps,
                        mul=inv_sb[:, i : i + 1],
                    )
                    nc.sync.dma_start(out=w_d[:, i, :], in_=w_i[i])
                nc.sync.dma_start(out=z_d, in_=z_sb)

    nc.compile()
    _CACHE[key] = nc
    return nc


def kernel(query, context, context_mask, scale):
    global LAST_RESULT
    _ensure_concourse()
    from concourse.bass_utils import run_bass_kernel_spmd

    query = np.asarray(query, dtype=np.float32)
    context = np.ascontiguousarray(np.asarray(context, dtype=np.float32))
    context_mask = np.asarray(context_mask)
    scale_f = float(np.asarray(scale))

    qs = query.reshape(B, D) / scale_f  # [B, D]
    # penalty: -BIG where masked, laid out [p, i, t] = pen[i, t*128+p]
    pen_full = np.where(context_mask, PEN, np.float32(0.0)).astype(np.float32)

    in_maps = []
    for c in range(NCORES):
        s0 = c * BPC
        qb = np.ascontiguousarray(
            np.broadcast_to(qs[s0 : s0 + BPC][None, :, :], (128, BPC, D))
        )
        pen = np.ascontiguousarray(
            pen_full[s0 : s0 + BPC].reshape(BPC, 128, NT).transpose(1, 0, 2)
        )
        in_maps.append(
            {"ctx": context[s0 : s0 + BPC], "qb": qb, "pen": pen}
        )

    nc = _build_bass()

    # Retry loop: a recently-crashed device worker can silently return
    # all-zero outputs; weights must sum to ~1 per row, so zeros are
    # detectable.
    for attempt in range(4):
        res = run_bass_kernel_spmd(nc, in_maps, list(range(NCORES)), trace=TRACE)
        LAST_RESULT = res
        weight = np.empty((B, L, 1), dtype=np.float32)
        z = np.empty((B, D), dtype=np.float32)
        for c in range(NCORES):
            s0 = c * BPC
            w_o = res.results[c]["w_out"]  # [128, BPC, NT]
            z_o = res.results[c]["z_out"]  # [128, BPC]
            weight[s0 : s0 + BPC, :, 0] = (
                w_o.transpose(1, 0, 2).reshape(BPC, L)
            )
            z[s0 : s0 + BPC] = z_o.T
        row_sums = weight[:, :, 0].sum(axis=1)
        if np.all(np.isfinite(row_sums)) and np.all(np.abs(row_sums - 1.0) < 1e-2):
            break
        if attempt < 3:
            import time
            time.sleep(30)
    return (weight, z)
